# revision 5
# baseline (speedup 1.0000x reference)
"""Trainium2 Bass kernel for nn_MultiHeadAttention_84576495993495.

Key observation: the reference module's output einsum is
    out = einsum('bhqk,bhvo->bhvo', attn, v)
which contracts softmax(attn) over BOTH q and k. Every softmax row sums
to 1, so sum_{q,k} attn == S (= 2048) and the whole attention block
collapses to out == S * v. Hence

    reference(x, ...) == ((x @ Wv.T + bv) * S) @ Wp.T + bp
                      ==  x @ M + c
with
    M = S * Wv.T @ Wp.T          (folded on host, fp64 then cast)
    c = S * Wp @ bv + bp

(Verified vs the jax reference: rel Frobenius err ~4.6e-7, i.e. fp32
noise.)  The device work is the data-dependent GEMM x @ M + c, sharded
data-parallel over the 8192 rows: 1024 rows per NeuronCore.

Per core: y_rows = xT.T @ M + c via 8 r-tiles x 2 n-chunks x 8 k-step
PSUM-accumulated fp32 matmuls. The x shard is pre-transposed on the
host (layout prep) because the TensorE contracts over the partition
dim; fp32 has no DMA-transpose path on TRN2.
"""

import os
from functools import lru_cache

import numpy as np

import concourse.bass as bass
import concourse.mybir as mybir
import concourse.tile as tile
from concourse import bacc
from concourse.bass_utils import run_bass_kernel_spmd

N_CORES = 8
P = 128
D = 1024                       # model dim (= SLICE_SIZE)
B, S = 4, 2048
R_TOTAL = B * S                # 8192 rows
R_CORE = R_TOTAL // N_CORES    # 1024 rows per core
K_TILES = D // P               # 8
R_TILES = R_CORE // P          # 8
N_CHUNK = 512                  # fp32 moving-operand max / one PSUM bank
N_CHUNKS = D // N_CHUNK        # 2
SCALE = float(S)               # module divides scores by sqrt(D); softmax sums to 1

# matmul input dtype: float32 (exact, 4 cyc/row) or float32r (1 cyc/row at N>=256)
MM_DTYPE_NAME = os.environ.get("KMM_DTYPE", "float32")


@lru_cache(maxsize=4)
def _build_nc(mm_dtype_name: str, loop_iters: int | None = None, sched: str | None = None):
    """loop_iters: when set, wrap the compute body in a tc.For_i hardware
    loop (inputs loaded once) — used by the benchmark harness to measure
    steady-state per-iteration device time without NTFF profiling."""
    if sched is None:
        sched = os.environ.get("KMM_SCHED", "v2")
    mm_dt = getattr(mybir.dt, mm_dtype_name)
    nc = bacc.Bacc(None, target_bir_lowering=False)

    xT = nc.dram_tensor("xT", [D, R_CORE], mm_dt, kind="ExternalInput")
    Mw = nc.dram_tensor("Mw", [D, D], mm_dt, kind="ExternalInput")
    cb = nc.dram_tensor("cb", [P, D], mybir.dt.float32, kind="ExternalInput")
    y = nc.dram_tensor("y", [R_CORE, D], mybir.dt.float32, kind="ExternalOutput")

    xT_t = xT.rearrange("(ko p) r -> p ko r", p=P)   # [128, 8, 1024]
    Mw_t = Mw.rearrange("(ko p) n -> p ko n", p=P)   # [128, 8, 1024]

    with tile.TileContext(nc) as tc:
        with (
            tc.tile_pool(name="wpool", bufs=1) as wpool,
            tc.tile_pool(name="opool", bufs=4) as opool,
            tc.tile_pool(
                name="pspool", bufs=(8 if sched == "v2" else 4), space="PSUM"
            ) as pspool,
        ):
            xT_sb = wpool.tile([P, K_TILES, R_CORE], mm_dt, tag="xT_sb")
            M_sb = wpool.tile([P, K_TILES, D], mm_dt, tag="M_sb")
            cb_sb = wpool.tile([P, D], mybir.dt.float32, tag="cb_sb")

            nc.sync.dma_start(cb_sb[:], cb[:])
            if sched == "v1":
                for k in range(K_TILES):
                    nc.sync.dma_start(xT_sb[:, k], xT_t[:, k])
                    nc.sync.dma_start(M_sb[:, k], Mw_t[:, k])
            else:
                # Finer DMA granularity so the first k-step's operands land
                # fast: interleave xT[k] with the M[k, nch] quarter-tiles in
                # consumption order.
                for k in range(K_TILES):
                    nc.sync.dma_start(xT_sb[:, k], xT_t[:, k])
                    for nch in range(N_CHUNKS):
                        nc.sync.dma_start(
                            M_sb[:, k, bass.ts(nch, N_CHUNK)],
                            Mw_t[:, k, bass.ts(nch, N_CHUNK)],
                        )

            def emit_group_tail(r, nch, ps):
                out_sb = opool.tile([P, N_CHUNK], mybir.dt.float32, tag="out_sb")
                nc.vector.tensor_add(
                    out_sb[:], ps[:], cb_sb[:, bass.ts(nch, N_CHUNK)]
                )
                nc.sync.dma_start(
                    y[bass.ts(r, P), bass.ts(nch, N_CHUNK)], out_sb[:]
                )

            def body_v1():
                for r in range(R_TILES):
                    for nch in range(N_CHUNKS):
                        ps = pspool.tile([P, N_CHUNK], mybir.dt.float32, tag="ps")
                        for k in range(K_TILES):
                            nc.tensor.matmul(
                                ps[:],
                                xT_sb[:, k, bass.ts(r, P)],
                                M_sb[:, k, bass.ts(nch, N_CHUNK)],
                                start=(k == 0),
                                stop=(k == K_TILES - 1),
                            )
                        emit_group_tail(r, nch, ps)

            def body_v2():
                # k-major within each n-chunk half: 8 live PSUM banks; the
                # PE unblocks on (xT[k], M[k,nch]) pairs (~768 KB) instead
                # of the whole 8.5 MB working set.
                for nch in range(N_CHUNKS):
                    groups = [
                        pspool.tile([P, N_CHUNK], mybir.dt.float32, tag="ps")
                        for _ in range(R_TILES)
                    ]
                    for k in range(K_TILES):
                        for r in range(R_TILES):
                            nc.tensor.matmul(
                                groups[r][:],
                                xT_sb[:, k, bass.ts(r, P)],
                                M_sb[:, k, bass.ts(nch, N_CHUNK)],
                                start=(k == 0),
                                stop=(k == K_TILES - 1),
                            )
                    for r in range(R_TILES):
                        emit_group_tail(r, nch, groups[r])

            body = body_v1 if sched == "v1" else body_v2
            if loop_iters is None:
                body()
            else:
                with tc.For_i(0, loop_iters, 1):
                    body()
    nc.compile()
    return nc


def _host_prep(x, Wv, bv, Wp, bp):
    X = np.ascontiguousarray(x, dtype=np.float32).reshape(R_TOTAL, D)
    M = (SCALE * (Wv.T.astype(np.float64) @ Wp.T.astype(np.float64))).astype(
        np.float32
    )
    c = (SCALE * (Wp.astype(np.float64) @ bv.astype(np.float64)) + bp).astype(
        np.float32
    )
    cbt = np.ascontiguousarray(np.broadcast_to(c, (P, D)))
    in_maps = []
    for i in range(N_CORES):
        shard = X[i * R_CORE : (i + 1) * R_CORE]
        in_maps.append(
            {
                "xT": np.ascontiguousarray(shard.T),
                "Mw": M,
                "cb": cbt,
            }
        )
    return in_maps


def kernel(x, Wq, bq, Wk, bk, Wv, bv, Wp, bp):
    nc = _build_nc(MM_DTYPE_NAME)
    in_maps = _host_prep(x, Wv, bv, Wp, bp)
    res = run_bass_kernel_spmd(nc, in_maps, core_ids=list(range(N_CORES)))
    y = np.concatenate([r["y"] for r in res.results], axis=0)
    return y.reshape(B, S, D)


# revision 8
# speedup vs baseline: 1.7949x; 1.7949x over previous
"""Trainium2 Bass kernel for nn_MultiHeadAttention_84576495993495.

Key observation: the reference module's output einsum is
    out = einsum('bhqk,bhvo->bhvo', attn, v)
which contracts softmax(attn) over BOTH q and k. Every softmax row sums
to 1, so sum_{q,k} attn == S (= 2048) and the whole attention block
collapses to out == S * v. Hence

    reference(x, ...) == ((x @ Wv.T + bv) * S) @ Wp.T + bp
                      ==  x @ M + c
with
    M = S * Wv.T @ Wp.T          (folded on host in fp64, then split)
    c = S * Wp @ bv + bp

(Verified vs the jax reference: rel Frobenius err ~3.6e-7 = fp32 noise.)

Device work: the data-dependent GEMM y = x @ M + c, sharded
data-parallel over the 8192 rows -> 1024 rows per NeuronCore.

Precision strategy: TensorE native fp32 matmul runs at 4 cyc/row (and
measures ~2x worse than that on HW); fp16 runs at 1 cyc/row.  So x and
M are each split into a high + low fp16 pair (x = xh + xl, M = Mh + Ml,
each pair exact to ~2^-22 relative) and the GEMM is computed as three
fp16 passes accumulated in the same fp32 PSUM group:
    y = xh@Mh + xh@Ml + xl@Mh  (+ c)
The dropped xl@Ml term is ~2^-22 relative -- below fp32 round-off for
this problem.  CPU-verified: rel err 3.56e-7, identical to a pure-fp32
evaluation of the same GEMM.

Layout: the TensorE contracts over the partition dim, so the x shard is
fed pre-transposed (host-side layout prep; fp32/fp16 DMA-transpose of
the activation on-device is not worth it here).  Per n-chunk half, the
schedule is k-major across 8 live PSUM banks so the PE only ever waits
for one (x[k], M[k]) tile pair (~384 KB) instead of the whole working
set.
"""

import os
from functools import lru_cache

import numpy as np

import concourse.bass as bass
import concourse.mybir as mybir
import concourse.tile as tile
from concourse import bacc
from concourse.bass_utils import run_bass_kernel_spmd

N_CORES = 8
P = 128
D = 1024                       # model dim (= SLICE_SIZE)
B, S = 4, 2048
R_TOTAL = B * S                # 8192 rows
R_CORE = R_TOTAL // N_CORES    # 1024 rows per core
K_TILES = D // P               # 8
R_TILES = R_CORE // P          # 8
N_CHUNK = 512                  # one PSUM bank / fp32 moving-operand max
N_CHUNKS = D // N_CHUNK        # 2
SCALE = float(S)               # sum over q,k of softmax rows == S

# "fp16x3" (default) | "float32" | "float32r"
MM_MODE = os.environ.get("KMM_DTYPE", "fp16x3")


@lru_cache(maxsize=4)
def _build_nc(mode: str, loop_iters: int | None = None):
    """loop_iters: when set, wrap the compute body in a tc.For_i hardware
    loop (inputs loaded once) -- used by the benchmark harness to measure
    steady-state per-iteration device time without NTFF profiling."""
    split = mode == "fp16x3"
    mm_dt = mybir.dt.float16 if split else getattr(mybir.dt, mode)
    nc = bacc.Bacc(None, target_bir_lowering=False)

    if split:
        x_names, m_names = ["xh", "xl"], ["Mh", "Ml"]
    else:
        x_names, m_names = ["xh"], ["Mh"]
    x_dram = [
        nc.dram_tensor(n, [D, R_CORE], mm_dt, kind="ExternalInput") for n in x_names
    ]
    m_dram = [nc.dram_tensor(n, [D, D], mm_dt, kind="ExternalInput") for n in m_names]
    cb = nc.dram_tensor("cb", [P, D], mybir.dt.float32, kind="ExternalInput")
    y = nc.dram_tensor("y", [R_CORE, D], mybir.dt.float32, kind="ExternalOutput")

    x_t = [t.rearrange("(ko p) r -> p ko r", p=P) for t in x_dram]   # [128, 8, 1024]
    m_t = [t.rearrange("(ko p) n -> p ko n", p=P) for t in m_dram]   # [128, 8, 1024]

    # (x operand, M operand) per accumulation pass; the xl@Ml term is dropped.
    passes = [(0, 0), (0, 1), (1, 0)] if split else [(0, 0)]

    with tile.TileContext(nc) as tc:
        with (
            tc.tile_pool(name="wpool", bufs=1) as wpool,
            tc.tile_pool(name="opool", bufs=4) as opool,
            tc.tile_pool(name="pspool", bufs=8, space="PSUM") as pspool,
        ):
            x_sb = [
                wpool.tile([P, K_TILES, R_CORE], mm_dt, tag=f"x_sb{i}", name=f"x_sb{i}")
                for i in range(len(x_dram))
            ]
            m_sb = [
                wpool.tile([P, K_TILES, D], mm_dt, tag=f"m_sb{i}", name=f"m_sb{i}")
                for i in range(len(m_dram))
            ]
            cb_sb = wpool.tile([P, D], mybir.dt.float32, tag="cb_sb")

            nc.sync.dma_start(cb_sb[:], cb[:])
            # Load in pass-0 consumption order first (xh, Mh), then the
            # low halves; per-k granularity so the PE can chase the stream.
            for i in range(len(x_dram)):
                for k in range(K_TILES):
                    nc.sync.dma_start(x_sb[i][:, k], x_t[i][:, k])
                    for nch in range(N_CHUNKS):
                        nc.sync.dma_start(
                            m_sb[i][:, k, bass.ts(nch, N_CHUNK)],
                            m_t[i][:, k, bass.ts(nch, N_CHUNK)],
                        )

            def body():
                for nch in range(N_CHUNKS):
                    groups = [
                        pspool.tile([P, N_CHUNK], mybir.dt.float32, tag="ps", name="ps")
                        for _ in range(R_TILES)
                    ]
                    n_acc = len(passes) * K_TILES
                    step = 0
                    for xi, mi in passes:
                        for k in range(K_TILES):
                            for r in range(R_TILES):
                                nc.tensor.matmul(
                                    groups[r][:],
                                    x_sb[xi][:, k, bass.ts(r, P)],
                                    m_sb[mi][:, k, bass.ts(nch, N_CHUNK)],
                                    start=(step == 0),
                                    stop=(step == n_acc - 1),
                                )
                            step += 1
                    for r in range(R_TILES):
                        out_sb = opool.tile(
                            [P, N_CHUNK], mybir.dt.float32, tag="out_sb"
                        )
                        nc.vector.tensor_add(
                            out_sb[:], groups[r][:], cb_sb[:, bass.ts(nch, N_CHUNK)]
                        )
                        nc.sync.dma_start(
                            y[bass.ts(r, P), bass.ts(nch, N_CHUNK)], out_sb[:]
                        )

            if loop_iters is None:
                body()
            else:
                with tc.For_i(0, loop_iters, 1):
                    body()
    nc.compile()
    return nc


def _host_prep(x, Wv, bv, Wp, bp, mode=None):
    mode = mode or MM_MODE
    X = np.ascontiguousarray(x, dtype=np.float32).reshape(R_TOTAL, D)
    M64 = SCALE * (Wv.T.astype(np.float64) @ Wp.T.astype(np.float64))
    c = (SCALE * (Wp.astype(np.float64) @ bv.astype(np.float64)) + bp).astype(
        np.float32
    )
    cbt = np.ascontiguousarray(np.broadcast_to(c, (P, D)))

    if mode == "fp16x3":
        Mh = M64.astype(np.float16)
        Ml = (M64 - Mh.astype(np.float64)).astype(np.float16)
        m_arrs = {"Mh": Mh, "Ml": Ml}
    else:
        m_arrs = {"Mh": M64.astype(np.float32)}

    in_maps = []
    for i in range(N_CORES):
        shard_t = np.ascontiguousarray(X[i * R_CORE : (i + 1) * R_CORE].T)
        im = dict(m_arrs)
        im["cb"] = cbt
        if mode == "fp16x3":
            xh = shard_t.astype(np.float16)
            xl = (shard_t - xh.astype(np.float32)).astype(np.float16)
            im["xh"] = xh
            im["xl"] = xl
        else:
            im["xh"] = shard_t
        in_maps.append(im)
    return in_maps


def kernel(x, Wq, bq, Wk, bk, Wv, bv, Wp, bp):
    x, Wv, bv, Wp, bp = (np.asarray(a) for a in (x, Wv, bv, Wp, bp))
    nc = _build_nc(MM_MODE)
    in_maps = _host_prep(x, Wv, bv, Wp, bp)
    res = run_bass_kernel_spmd(nc, in_maps, core_ids=list(range(N_CORES)))
    y = np.concatenate([r["y"] for r in res.results], axis=0)
    return y.reshape(B, S, D)
